# revision 8
# baseline (speedup 1.0000x reference)
"""Cross-attention block on 8 Trainium2 NeuronCores.

Computes, per batch b:
    xn = LN(x); cn = LN(cond)
    q = xn @ Wq; k = cn @ Wk; v = cn @ Wv   (8 heads x 64)
    out = softmax(q k^T / sqrt(64)) v
    y  = LN(out @ Wo + bo + x)

Sharding: 8 cores = 4 batches x 2 query-row halves (data parallel over
(batch, query-block)).  Each core recomputes LN(cond)/K/V for its batch
(duplicated across the 2 cores of a batch) and produces a disjoint
[1024, 512] slice of the output, so no collectives are needed.

On-core layout: activations are kept transposed (features on SBUF
partitions, tokens on the free axis), so LN affine params become
per-partition scalars and the attention scores can be built directly in
S^T form (keys on partitions), which feeds the P@V matmul without any
transposes of the 16.8M-element probability matrix.  Softmax is computed
without max-subtraction (scores are ~N(0,1), |s| < ~7, exp is safe in
fp32) and the denominator comes from a ones-matmul on the tensor engine,
accumulated alongside O^T = V^T P^T.  Matmul inputs are bf16, all
accumulation fp32.
"""

import functools

import numpy as np

B, N, M = 4, 2048, 2048
DQ, DC = 512, 768
H, DH = 8, 64
INNER = H * DH  # 512
P = 128
NQ = N // 2  # query rows per core
EPS = 1e-5
N_CORES = 8

FC_X = DQ // P  # 4 feature chunks of x
FC_C = DC // P  # 6 feature chunks of cond
IC = INNER // P  # 4 inner chunks
TQ = NQ // P  # 8 query-token chunks per core
TK = M // P  # 16 key-token chunks
NT = NQ // 512  # 2 query column tiles (transposed layout)
KNT = M // 512  # 4 key column tiles


def _emit(tc, io):
    import concourse.bass as bass
    import concourse.mybir as mybir
    from concourse.masks import make_identity

    nc = tc.nc
    f32 = mybir.dt.float32
    bf16 = mybir.dt.bfloat16
    AF = mybir.ActivationFunctionType
    OP = mybir.AluOpType

    import contextlib

    ctx = contextlib.ExitStack()
    with ctx:
        singles = ctx.enter_context(tc.tile_pool(name="singles", bufs=1))
        wstage = ctx.enter_context(tc.tile_pool(name="wstage", bufs=2))
        work = ctx.enter_context(tc.tile_pool(name="work", bufs=3))
        stat = ctx.enter_context(tc.tile_pool(name="stat", bufs=4))
        cenp = ctx.enter_context(tc.tile_pool(name="cenp", bufs=6))
        ppool = ctx.enter_context(tc.tile_pool(name="ppool", bufs=8))
        ps = ctx.enter_context(tc.tile_pool(name="ps", bufs=2, space="PSUM"))

        # ---- constants -------------------------------------------------
        ident = singles.tile([P, P], bf16, name="ident")
        make_identity(nc, ident)
        eps_t = singles.tile([P, 1], f32, name="eps_t")
        nc.vector.memset(eps_t, EPS)
        ones_t = singles.tile([P, DH], bf16, name="ones_t")
        nc.vector.memset(ones_t, 1.0)

        def bcast_load(vec_ap, width, name):
            """[width] dram vector -> [128, width] sbuf tile (same row on
            every partition)."""
            t = singles.tile([P, width], f32, name=name)
            bc = bass.AP(
                tensor=vec_ap.tensor,
                offset=vec_ap.offset,
                ap=[[0, P]] + [list(a) for a in vec_ap.ap],
            )
            nc.gpsimd.dma_start(out=t, in_=bc)
            return t

        def strip_load(vec_ap, chunks, name):
            """[chunks*128] dram vector -> [128, chunks] sbuf (feature-on-
            partition layout)."""
            t = singles.tile([P, chunks], f32, name=name)
            nc.gpsimd.dma_start(out=t, in_=vec_ap.rearrange("(c p) -> p c", p=P))
            return t

        gx = strip_load(io["lnx_g"], FC_X, "gx")
        bx = strip_load(io["lnx_b"], FC_X, "bx")
        gc = strip_load(io["lnc_g"], FC_C, "gc")
        bc_ = strip_load(io["lnc_b"], FC_C, "bc")
        gf_bc = bcast_load(io["lnf_g"], DQ, "gf_bc")
        bf_bc = bcast_load(io["lnf_b"], DQ, "bf_bc")
        bo_bc = bcast_load(io["bo"], DQ, "bo_bc")

        # ---- weights: fp32 HBM -> bf16 SBUF, contraction on partitions --
        def load_weight(w_ap, din, name):
            kc = din // P
            stage = wstage.tile([P, kc, INNER], f32, tag="wstage", name=f"{name}_st")
            nc.gpsimd.dma_start(
                out=stage, in_=w_ap.rearrange("(ko p) i -> p ko i", p=P)
            )
            wb = singles.tile([P, kc, INNER], bf16, name=name)
            nc.vector.tensor_copy(out=wb, in_=stage)
            return wb

        wq_b = load_weight(io["Wq"], DQ, "wq_b")
        wk_b = load_weight(io["Wk"], DC, "wk_b")
        wv_b = load_weight(io["Wv"], DC, "wv_b")
        wo_b = load_weight(io["Wo"], INNER, "wo_b")

        # ---- persistent activations ------------------------------------
        xnT = singles.tile([P, FC_X, NQ], bf16, name="xnT")  # LN(x)^T
        cnT = singles.tile([P, FC_C, M], bf16, name="cnT")  # LN(cond)^T
        QT = singles.tile([P, IC, NQ], bf16, name="QT")  # (q*scale)^T
        KT = singles.tile([P, IC, M], bf16, name="KT")  # k^T
        V = singles.tile([P, TK, INNER], bf16, name="V")  # v (token layout)
        OT = singles.tile([P, IC, NQ], bf16, name="OT")  # attn out^T

        # ---- LayerNorm helper ------------------------------------------
        import math

        def ln_stats(x_t, width):
            fmax = math.gcd(512, width)
            nsub = width // fmax
            if nsub == 1:
                stats = stat.tile([P, 6], f32, tag="bnstats", name="stats")
                nc.vector.bn_stats(out=stats, in_=x_t)
            else:
                xr = x_t.rearrange("p (s f) -> p s f", f=fmax)
                stats = stat.tile([P, nsub, 6], f32, tag="bnstats", name="stats")
                for s in range(nsub):
                    nc.vector.bn_stats(out=stats[:, s], in_=xr[:, s])
            mv = stat.tile([P, 2], f32, tag="bnaggr", name="mv")
            nc.vector.bn_aggr(out=mv, in_=stats)
            std = stat.tile([P, 1], f32, tag="std", name="std")
            nc.scalar.activation(
                out=std, in_=mv[:, 1:2], func=AF.Sqrt, bias=eps_t, scale=1.0
            )
            rstd = stat.tile([P, 1], f32, tag="rstd", name="rstd")
            nc.vector.reciprocal(out=rstd, in_=std)
            return mv[:, 0:1], rstd

        # ---- phase 1/2: LN + transpose into feature-major layout -------
        def ln_transpose(src_ap, width, tchunks, g_strip, b_strip, dst):
            fc_n = width // P
            src = src_ap.rearrange("(t p) d -> p t d", p=P)
            for tg in range(tchunks // 4):
                cents = []
                for tl in range(4):
                    t = tg * 4 + tl
                    x_t = work.tile([P, width], f32, tag="xin", name="x_t")
                    nc.gpsimd.dma_start(out=x_t, in_=src[:, t])
                    mean, rstd = ln_stats(x_t, width)
                    cen = cenp.tile([P, width], bf16, tag="cen", name="cen")
                    nc.vector.tensor_scalar(
                        out=cen,
                        in0=x_t,
                        scalar1=mean,
                        scalar2=rstd,
                        op0=OP.subtract,
                        op1=OP.mult,
                    )
                    cents.append(cen)
                for fc in range(fc_n):
                    tp = ps.tile([P, 4, P], bf16, tag="st", name="tp")
                    for tl in range(4):
                        nc.tensor.transpose(
                            tp[:, tl], cents[tl][:, fc * P : (fc + 1) * P], ident
                        )
                    # dst = tp * g[fc] + b[fc]   (per-partition scalars)
                    nc.vector.tensor_scalar(
                        out=dst[:, fc, tg * 512 : (tg + 1) * 512],
                        in0=tp,
                        scalar1=g_strip[:, fc : fc + 1],
                        scalar2=b_strip[:, fc : fc + 1],
                        op0=OP.mult,
                        op1=OP.add,
                    )

        ln_transpose(io["x"], DQ, TQ, gx, bx, xnT)
        ln_transpose(io["cond"], DC, TK, gc, bc_, cnT)

        # ---- phase 3: projections --------------------------------------
        scale = float(DH) ** -0.5
        for m in range(IC):
            for nt in range(NT):
                ps_q = ps.tile([P, 512], f32, tag="acc", bufs=4, name="ps_q")
                for k in range(FC_X):
                    nc.tensor.matmul(
                        ps_q,
                        lhsT=wq_b[:, k, m * P : (m + 1) * P],
                        rhs=xnT[:, k, nt * 512 : (nt + 1) * 512],
                        start=(k == 0),
                        stop=(k == FC_X - 1),
                    )
                nc.vector.tensor_scalar(
                    out=QT[:, m, nt * 512 : (nt + 1) * 512],
                    in0=ps_q,
                    scalar1=scale,
                    scalar2=None,
                    op0=OP.mult,
                )
        for m in range(IC):
            for nt in range(KNT):
                ps_k = ps.tile([P, 512], f32, tag="acc", bufs=4, name="ps_k")
                for k in range(FC_C):
                    nc.tensor.matmul(
                        ps_k,
                        lhsT=wk_b[:, k, m * P : (m + 1) * P],
                        rhs=cnT[:, k, nt * 512 : (nt + 1) * 512],
                        start=(k == 0),
                        stop=(k == FC_C - 1),
                    )
                nc.vector.tensor_copy(
                    out=KT[:, m, nt * 512 : (nt + 1) * 512], in_=ps_k
                )
        for mc in range(TK):
            ps_v = ps.tile([P, 512], f32, tag="acc", bufs=4, name="ps_v")
            for k in range(FC_C):
                nc.tensor.matmul(
                    ps_v,
                    lhsT=cnT[:, k, mc * P : (mc + 1) * P],
                    rhs=wv_b[:, k, :],
                    start=(k == 0),
                    stop=(k == FC_C - 1),
                )
            nc.vector.tensor_copy(out=V[:, mc, :], in_=ps_v)

        # ---- phase 4: attention ----------------------------------------
        # Heads 2c / 2c+1 live in partitions 0:64 / 64:128 of chunk c, so
        # the two S^T matmuls of a pair run row-tiled concurrently and the
        # two O^T matmuls run col-tiled concurrently.
        for c in range(H // 2):
            hA, hB = 2 * c, 2 * c + 1
            for nt in range(NT):
                q_a = QT[0:64, c, nt * 512 : (nt + 1) * 512]
                q_b = QT[64:128, c, nt * 512 : (nt + 1) * 512]
                # One bank per head: head A accumulates in partitions 0:64
                # of its bank, head B in partitions 64:128 of another bank,
                # so the pair still runs col-tiled concurrently while each
                # bank holds a single accumulation group.
                ot_a = ps.tile([P, 512], f32, tag="acc", bufs=4, name="ot_a")
                ot_b = ps.tile([P, 512], f32, tag="acc", bufs=4, name="ot_b")
                den_a = ps.tile([P, 512], f32, tag="acc", bufs=4, name="den_a")
                den_b = ps.tile([P, 512], f32, tag="acc", bufs=4, name="den_b")

                def emit_pv(mg, pa, pb):
                    for j in range(2):
                        mc = mg * 2 + j
                        first = mc == 0
                        last = mc == TK - 1
                        nc.tensor.matmul(
                            ot_a[0:64, :],
                            lhsT=V[:, mc, hA * DH : (hA + 1) * DH],
                            rhs=pa[:, j],
                            start=first,
                            stop=last,
                        )
                        nc.tensor.matmul(
                            ot_b[64:128, :],
                            lhsT=V[:, mc, hB * DH : (hB + 1) * DH],
                            rhs=pb[:, j],
                            start=first,
                            stop=last,
                        )
                        nc.tensor.matmul(
                            den_a[0:64, :],
                            lhsT=ones_t,
                            rhs=pa[:, j],
                            start=first,
                            stop=last,
                        )
                        nc.tensor.matmul(
                            den_b[64:128, :],
                            lhsT=ones_t,
                            rhs=pb[:, j],
                            start=first,
                            stop=last,
                        )

                pend = None
                for mg in range(TK // 2):
                    st_a = ps.tile([P, 2, 512], f32, tag="st", name="st_a")
                    st_b = ps.tile([P, 2, 512], f32, tag="st", name="st_b")
                    for j in range(2):
                        mc = mg * 2 + j
                        nc.tensor.matmul(
                            st_a[:, j],
                            lhsT=KT[0:64, c, mc * P : (mc + 1) * P],
                            rhs=q_a,
                            start=True,
                            stop=True,
                        )
                        nc.tensor.matmul(
                            st_b[:, j],
                            lhsT=KT[64:128, c, mc * P : (mc + 1) * P],
                            rhs=q_b,
                            start=True,
                            stop=True,
                        )
                    pa = ppool.tile([P, 2, 512], bf16, tag="p", name="pa")
                    pb = ppool.tile([P, 2, 512], bf16, tag="p", name="pb")
                    nc.scalar.activation(out=pa, in_=st_a, func=AF.Exp)
                    nc.scalar.activation(out=pb, in_=st_b, func=AF.Exp)
                    if pend is not None:
                        emit_pv(*pend)
                    pend = (mg, pa, pb)
                emit_pv(*pend)

                recip = work.tile([P, 512], f32, tag="recip", name="recip")
                nc.vector.reciprocal(out=recip[0:64], in_=den_a[0:64])
                nc.vector.reciprocal(out=recip[64:128], in_=den_b[64:128])
                nc.vector.tensor_mul(
                    out=OT[0:64, c, nt * 512 : (nt + 1) * 512],
                    in0=ot_a[0:64],
                    in1=recip[0:64],
                )
                nc.vector.tensor_mul(
                    out=OT[64:128, c, nt * 512 : (nt + 1) * 512],
                    in0=ot_b[64:128],
                    in1=recip[64:128],
                )

        # ---- phase 5: Wo projection + residual + final LN --------------
        xr = io["x"].rearrange("(t p) d -> p t d", p=P)
        outr = io["out"].rearrange("(t p) d -> p t d", p=P)
        for t in range(TQ):
            y_ps = ps.tile([P, 512], f32, tag="acc", bufs=4, name="y_ps")
            for k in range(IC):
                nc.tensor.matmul(
                    y_ps,
                    lhsT=OT[:, k, t * P : (t + 1) * P],
                    rhs=wo_b[:, k, :],
                    start=(k == 0),
                    stop=(k == IC - 1),
                )
            x_t = work.tile([P, DQ], f32, tag="xres", name="x_t2")
            nc.gpsimd.dma_start(out=x_t, in_=xr[:, t])
            xb = work.tile([P, DQ], f32, tag="xb", name="xb")
            nc.gpsimd.tensor_add(out=xb, in0=x_t, in1=bo_bc)
            y1 = work.tile([P, DQ], f32, tag="y1", name="y1")
            nc.vector.tensor_add(out=y1, in0=y_ps, in1=xb)
            mean, rstd = ln_stats(y1, DQ)
            z = work.tile([P, DQ], f32, tag="z", name="z")
            nc.vector.tensor_scalar(
                out=z,
                in0=y1,
                scalar1=mean,
                scalar2=rstd,
                op0=OP.subtract,
                op1=OP.mult,
            )
            o_t = work.tile([P, DQ], f32, tag="otile", name="o_t")
            nc.vector.tensor_mul(out=o_t, in0=z, in1=gf_bc)
            nc.gpsimd.tensor_add(out=o_t, in0=o_t, in1=bf_bc)
            nc.gpsimd.dma_start(out=outr[:, t], in_=o_t)


@functools.cache
def _build_program():
    import concourse.bacc as bacc
    import concourse.mybir as mybir
    import concourse.tile as tile

    f32 = mybir.dt.float32
    nc = bacc.Bacc()
    io = {}
    io["x"] = nc.declare_dram_parameter("x", [NQ, DQ], f32, False)[:, :]
    io["cond"] = nc.declare_dram_parameter("cond", [M, DC], f32, False)[:, :]
    for name in ("lnx_g", "lnx_b"):
        io[name] = nc.declare_dram_parameter(name, [DQ], f32, False)[:]
    for name in ("lnc_g", "lnc_b"):
        io[name] = nc.declare_dram_parameter(name, [DC], f32, False)[:]
    io["Wq"] = nc.declare_dram_parameter("Wq", [DQ, INNER], f32, False)[:, :]
    io["Wk"] = nc.declare_dram_parameter("Wk", [DC, INNER], f32, False)[:, :]
    io["Wv"] = nc.declare_dram_parameter("Wv", [DC, INNER], f32, False)[:, :]
    io["Wo"] = nc.declare_dram_parameter("Wo", [INNER, DQ], f32, False)[:, :]
    for name in ("bo", "lnf_g", "lnf_b"):
        io[name] = nc.declare_dram_parameter(name, [DQ], f32, False)[:]
    io["out"] = nc.declare_dram_parameter("out", [NQ, DQ], f32, True)[:, :]

    with tile.TileContext(nc) as tc:
        _emit(tc, io)
    nc.compile()
    return nc


def _core_input_map(inputs, core):
    b, half = core // 2, core % 2
    m = {
        "x": np.ascontiguousarray(inputs["x"][b, half * NQ : (half + 1) * NQ]),
        "cond": np.ascontiguousarray(inputs["cond"][b]),
    }
    for name in (
        "lnx_g",
        "lnx_b",
        "lnc_g",
        "lnc_b",
        "Wq",
        "Wk",
        "Wv",
        "Wo",
        "bo",
        "lnf_g",
        "lnf_b",
    ):
        m[name] = np.asarray(inputs[name], dtype=np.float32)
    return m


TRACE = False
LAST_RESULTS = None


def kernel(**inputs):
    from concourse.bass_utils import run_bass_kernel_spmd

    global LAST_RESULTS
    nc = _build_program()
    in_maps = [_core_input_map(inputs, core) for core in range(N_CORES)]
    res = run_bass_kernel_spmd(
        nc,
        in_maps,
        list(range(N_CORES)),
        trace=TRACE,
        trace_cores=[0] if TRACE else None,
    )
    LAST_RESULTS = res
    out = np.empty((B, N, DQ), np.float32)
    for core in range(N_CORES):
        b, half = core // 2, core % 2
        out[b, half * NQ : (half + 1) * NQ] = res.results[core]["out"]
    return out
